# Initial kernel scaffold
#
"""BiAffineAttention Trainium2 kernel (8-core data-parallel over batch).

Math per batch element b (from the reference):
  Hah = relu(H @ W_ah^T + b_ah)        [L, DA]   (arc head)
  Had = relu(H @ W_ad^T + b_ad)        [L, DA]   (arc dep)
  Hrh = relu(H @ W_rh^T + b_rh)        [L, DR]   (rel head)
  Hrd = relu(H @ W_rd^T + b_rd)        [L, DR]   (rel dep)
  S_arc[l, m] = sum_e (Had @ W_arc + b_arc)[l, e] * Hah[m, e]
  preds = argmax_m S_arc
  Hg[l] = Hrh[preds[l]]
  S_rel[l, r] = sum_ij Hg[l,i] U[i,r,j] Hrd[l,j] + (Hg+Hrd)@W_rel + b_rel

On-chip layout strategy (everything chained through transposed activations):
  - H^T [HID2+1, L] built once per b via PE transposes (last row = ones for
    folding projection biases into the matmul via augmented weights).
  - Projections computed as P^T = W_aug^T.T @ H^T (transposed layout) except
    Hrh which is needed in natural layout (lhsT for the gather matmul).
  - A'^T = W_arc_aug.T @ Had^T_aug (b_arc folded via ones row).
  - S_arc tile [l, m] = A'^T.T @ Hah^T; argmax over free dim (m).
  - Gather Hg^T = Hrh_nat.T @ OneHotT where OneHotT[m, l] = (preds[l] == m),
    built from a broadcast matmul of the preds row + iota compare.
  - G = Hg_aug^T.T @ [U_flat | W_rel; 0 | b_rel]  (K = DR+1, ones row in Hg)
  - S_rel = segmented_reduce_j(G[:, (r, j)] * Hrd_nat[:, j]) + G_wrel_block
            + Hrd @ W_rel (accumulated into the same PSUM block).
"""

import os
import sys
from contextlib import ExitStack

import numpy as np

import concourse.bass as bass
import concourse.tile as tile
from concourse import mybir
from concourse.bass_utils import run_bass_kernel_spmd
from concourse.masks import make_identity

F32 = mybir.dt.float32
F32R = mybir.dt.float32r
I32 = mybir.dt.int32
U32 = mybir.dt.uint32

B, L, HID2 = 64, 512, 800
DA, DR, NR = 500, 100, 46
NCORES = 8
BSH = B // NCORES  # batch shard per core

KH = HID2 + 1      # augmented contraction dim for projections (ones row)
KA = DA + 1        # augmented contraction dim for W_arc
KG = DR + 1        # augmented contraction dim for the G matmul
NG = NR * DR + NR  # 4646: [U_flat | W_rel(+b_rel)] columns

# dtype config: float32r streams fp32 bits through the PE at ~4x the rate of
# plain fp32 (which needs 2 half-speed passes). Numerics validated on HW.
USE_F32R_PROJ = True   # projections + A + S_arc (argmax-critical chain)
USE_F32R_G = True      # G matmul + gather matmul

_chunks_cache = {}


def _chunks(total, step=128):
    key = (total, step)
    if key not in _chunks_cache:
        out = []
        o = 0
        while o < total:
            out.append((o, min(step, total - o)))
            o += step
        _chunks_cache[key] = out
    return _chunks_cache[key]


def _mm_dt(ap, use_f32r):
    return ap.bitcast(F32R) if use_f32r else ap


def build_nc():
    nc = bass.Bass()

    H = nc.declare_dram_parameter("H", [BSH, L, HID2], F32, isOutput=False)
    wahT = nc.declare_dram_parameter("wahT", [KH, DA], F32, isOutput=False)
    wadT = nc.declare_dram_parameter("wadT", [KH, DA], F32, isOutput=False)
    wrhT = nc.declare_dram_parameter("wrhT", [KH, DR], F32, isOutput=False)
    wrdT = nc.declare_dram_parameter("wrdT", [KH, DR], F32, isOutput=False)
    warc = nc.declare_dram_parameter("warc", [KA, DA], F32, isOutput=False)
    uaug = nc.declare_dram_parameter("uaug", [KG, NG], F32, isOutput=False)
    wrel = nc.declare_dram_parameter("wrel", [DR, NR], F32, isOutput=False)

    sarc_o = nc.declare_dram_parameter("S_arc", [BSH, L, L], F32, isOutput=True)
    srel_o = nc.declare_dram_parameter("S_rel", [BSH, L, NR], F32, isOutput=True)
    preds_o = nc.declare_dram_parameter("preds", [BSH, L], I32, isOutput=True)

    with ExitStack() as ctx, tile.TileContext(nc) as tc:
        consts = ctx.enter_context(tc.tile_pool(name="consts", bufs=1))
        wpool = ctx.enter_context(tc.tile_pool(name="wpool", bufs=1))
        hstage = ctx.enter_context(tc.tile_pool(name="hstage", bufs=2))
        htp = ctx.enter_context(tc.tile_pool(name="htp", bufs=2))
        projp = ctx.enter_context(tc.tile_pool(name="projp", bufs=2))
        projp1 = ctx.enter_context(tc.tile_pool(name="projp1", bufs=1))
        workp = ctx.enter_context(tc.tile_pool(name="workp", bufs=2))
        workp1 = ctx.enter_context(tc.tile_pool(name="workp1", bufs=1))
        outp = ctx.enter_context(tc.tile_pool(name="outp", bufs=2))
        ppA = ctx.enter_context(tc.tile_pool(name="ppA", bufs=3, space="PSUM"))
        ppG = ctx.enter_context(tc.tile_pool(name="ppG", bufs=2, space="PSUM"))
        ppT = ctx.enter_context(tc.tile_pool(name="ppT", bufs=2, space="PSUM"))
        ppS = ctx.enter_context(tc.tile_pool(name="ppS", bufs=1, space="PSUM"))

        identity = consts.tile([128, 128], F32)
        make_identity(nc, identity)
        iota_i = consts.tile([128, 1], I32)
        nc.gpsimd.iota(iota_i, pattern=[[0, 1]], base=0, channel_multiplier=1)
        iota_f = consts.tile([128, 1], F32)
        nc.vector.tensor_copy(iota_f, iota_i)
        ones_col = consts.tile([1, 128], F32)
        nc.vector.memset(ones_col, 1.0)

        # -------- weights (resident) --------
        w_ahT = wpool.tile([128, 7, DA], F32)
        w_adT = wpool.tile([128, 7, DA], F32)
        w_rhT = wpool.tile([128, 7, DR], F32)
        w_rdT = wpool.tile([128, 7, DR], F32)
        for k, (o, sz) in enumerate(_chunks(KH)):
            nc.sync.dma_start(out=w_ahT[0:sz, k, :], in_=wahT[o : o + sz, :])
            nc.sync.dma_start(out=w_adT[0:sz, k, :], in_=wadT[o : o + sz, :])
            nc.sync.dma_start(out=w_rhT[0:sz, k, :], in_=wrhT[o : o + sz, :])
            nc.sync.dma_start(out=w_rdT[0:sz, k, :], in_=wrdT[o : o + sz, :])
        w_arc = wpool.tile([128, 4, DA], F32)
        for k, (o, sz) in enumerate(_chunks(KA)):
            nc.sync.dma_start(out=w_arc[0:sz, k, :], in_=warc[o : o + sz, :])
        u_aug = wpool.tile([KG, NG], F32)
        nc.sync.dma_start(out=u_aug, in_=uaug)
        w_rel = wpool.tile([DR, NR], F32)
        nc.sync.dma_start(out=w_rel, in_=wrel)

        relu = mybir.ActivationFunctionType.Relu

        for b in range(BSH):
            # -------- stage 1: load H[b], build H^T (+ones row) --------
            ht = htp.tile([128, 7, L], F32, tag="ht")
            for lt in range(4):
                hn = hstage.tile([128, HID2], F32, tag="hn")
                nc.sync.dma_start(out=hn, in_=H[b, 128 * lt : 128 * (lt + 1), :])
                for k, (o, sz) in enumerate(_chunks(HID2)):
                    ptr = ppT.tile([128, 128], F32, tag="ptr")
                    nc.tensor.transpose(ptr[0:sz, :], hn[:, o : o + sz], identity)
                    nc.scalar.copy(
                        out=ht[0:sz, k, 128 * lt : 128 * (lt + 1)], in_=ptr[0:sz, :]
                    )
            nc.vector.memset(ht[32:33, 6, :], 1.0)  # ones row (d = 800)

            # -------- stage 2: projections --------
            hahT = projp.tile([128, 4, L], F32, tag="hahT")
            hadT = projp.tile([128, 4, L], F32, tag="hadT")
            for w_sb, dst in ((w_ahT, hahT), (w_adT, hadT)):
                for m, (mo, msz) in enumerate(_chunks(DA)):
                    pr = ppA.tile([128, L], F32, tag="pA")
                    for k, (o, sz) in enumerate(_chunks(KH)):
                        nc.tensor.matmul(
                            pr[0:msz, :],
                            _mm_dt(w_sb[0:sz, k, mo : mo + msz], USE_F32R_PROJ),
                            _mm_dt(ht[0:sz, k, :], USE_F32R_PROJ),
                            start=(k == 0),
                            stop=(k == 6),
                        )
                    nc.scalar.activation(dst[0:msz, m, :], pr[0:msz, :], relu)
            # ones row for the W_arc bias fold: global row 500 = chunk 3 row 116
            nc.vector.memset(hadT[116:117, 3, :], 1.0)

            # Hrd^T [DR, L]
            hrdT = projp.tile([DR, L], F32, tag="hrdT")
            prd = ppA.tile([128, L], F32, tag="pA")
            for k, (o, sz) in enumerate(_chunks(KH)):
                nc.tensor.matmul(
                    prd[0:DR, :],
                    _mm_dt(w_rdT[0:sz, k, :], USE_F32R_PROJ),
                    _mm_dt(ht[0:sz, k, :], USE_F32R_PROJ),
                    start=(k == 0),
                    stop=(k == 6),
                )
            nc.scalar.activation(hrdT, prd[0:DR, :], relu)

            # Hrh natural [l, i] per l-tile
            hrh = projp.tile([128, 4, DR], F32, tag="hrh")
            for lt in range(4):
                prh = ppS.tile([128, DR], F32, tag="pS")
                for k, (o, sz) in enumerate(_chunks(HID2)):
                    nc.tensor.matmul(
                        prh,
                        _mm_dt(ht[0:sz, k, 128 * lt : 128 * (lt + 1)], USE_F32R_PROJ),
                        _mm_dt(w_rhT[0:sz, k, :], USE_F32R_PROJ),
                        start=(k == 0),
                        stop=(k == 6),
                    )
                nc.scalar.activation(hrh[:, lt, :], prh, relu)

            # Hrd natural via PE transpose of Hrd^T
            hrdn = projp.tile([128, 4, DR], F32, tag="hrdn")
            for lt in range(4):
                ptr2 = ppT.tile([128, 128], F32, tag="ptr")
                nc.tensor.transpose(
                    ptr2[:, 0:DR], hrdT[:, 128 * lt : 128 * (lt + 1)], identity[0:DR, 0:DR]
                )
                nc.scalar.copy(out=hrdn[:, lt, :], in_=ptr2[:, 0:DR])

            # -------- stage 3: A'^T = W_arc_aug.T @ Had^T_aug --------
            aT = projp1.tile([128, 4, L], F32, tag="aT")
            for m, (mo, msz) in enumerate(_chunks(DA)):
                pa = ppA.tile([128, L], F32, tag="pA")
                for k, (o, sz) in enumerate(_chunks(KA)):
                    nc.tensor.matmul(
                        pa[0:msz, :],
                        _mm_dt(w_arc[0:sz, k, mo : mo + msz], USE_F32R_PROJ),
                        _mm_dt(hadT[0:sz, k, :], USE_F32R_PROJ),
                        start=(k == 0),
                        stop=(k == 3),
                    )
                nc.scalar.copy(out=aT[0:msz, m, :], in_=pa[0:msz, :])

            # -------- stage 4: S_arc + argmax --------
            sarc_sb = outp.tile([128, 4, L], F32, tag="sarc")
            mx = workp1.tile([128, 4, 8], F32, tag="mx")
            mi = workp1.tile([128, 4, 8], U32, tag="mi")
            pred_f = workp1.tile([128, 4], F32, tag="predf")
            pred_i = workp1.tile([128, 4], I32, tag="predi")
            for lt in range(4):
                ps = ppA.tile([128, L], F32, tag="pA")
                for k, (o, sz) in enumerate(_chunks(DA)):
                    nc.tensor.matmul(
                        ps,
                        _mm_dt(aT[0:sz, k, 128 * lt : 128 * (lt + 1)], USE_F32R_PROJ),
                        _mm_dt(hahT[0:sz, k, :], USE_F32R_PROJ),
                        start=(k == 0),
                        stop=(k == 3),
                    )
                nc.scalar.copy(out=sarc_sb[:, lt, :], in_=ps)
                nc.sync.dma_start(
                    out=sarc_o[b, 128 * lt : 128 * (lt + 1), :], in_=sarc_sb[:, lt, :]
                )
                nc.vector.max(out=mx[:, lt, :], in_=sarc_sb[:, lt, :])
                nc.vector.max_index(mi[:, lt, :], mx[:, lt, :], sarc_sb[:, lt, :])
                nc.vector.tensor_copy(pred_i[:, lt : lt + 1], mi[:, lt, 0:1])
                nc.vector.tensor_copy(pred_f[:, lt : lt + 1], mi[:, lt, 0:1])
                nc.sync.dma_start(
                    out=preds_o[b, 128 * lt : 128 * (lt + 1)],
                    in_=pred_i[:, lt : lt + 1],
                )

            # -------- stage 5: gather Hg^T via one-hot matmul --------
            head_row = workp1.tile([1, L], F32, tag="hrow")
            for lt in range(4):
                prow = ppS.tile([1, 128], F32, tag="pS")
                nc.tensor.transpose(prow, pred_f[:, lt : lt + 1], identity)
                nc.scalar.copy(
                    out=head_row[0:1, 128 * lt : 128 * (lt + 1)], in_=prow
                )
            pbc = ppA.tile([128, L], F32, tag="pA")
            nc.tensor.matmul(pbc, ones_col, head_row, start=True, stop=True)
            oh = workp1.tile([128, 4, L], F32, tag="oh")
            for mt in range(4):
                nc.vector.tensor_scalar(
                    out=oh[:, mt, :],
                    in0=pbc,
                    scalar1=iota_f,
                    scalar2=float(128 * mt),
                    op0=mybir.AluOpType.subtract,
                    op1=mybir.AluOpType.is_equal,
                )
            phg = ppS.tile([DR, L], F32, tag="pS")
            for mt in range(4):
                nc.tensor.matmul(
                    phg,
                    _mm_dt(hrh[:, mt, :], USE_F32R_G),
                    _mm_dt(oh[:, mt, :], USE_F32R_G),
                    start=(mt == 0),
                    stop=(mt == 3),
                )
            hgT = workp1.tile([KG, L], F32, tag="hgT")
            nc.scalar.copy(out=hgT[0:DR, :], in_=phg)
            nc.vector.memset(hgT[DR : DR + 1, :], 1.0)

            # -------- stage 6: G matmul + bilinear reduce + S_rel --------
            srel_sb = outp.tile([128, 4, NR], F32, tag="srel")
            for lt in range(4):
                prod = workp1.tile([128, NR, DR], F32, tag="prod")
                pg_last = None
                for c in range(10):
                    n0 = 500 * c
                    nw = 500 if c < 9 else NG - 4500  # 146 for the last chunk
                    pg = ppG.tile([128, 500], F32, tag="pg")
                    nc.tensor.matmul(
                        pg[:, 0:nw],
                        _mm_dt(hgT[:, 128 * lt : 128 * (lt + 1)], USE_F32R_G),
                        _mm_dt(u_aug[:, n0 : n0 + nw], USE_F32R_G),
                        start=True,
                        stop=(c < 9),
                    )
                    if c == 9:
                        # accumulate Hrd @ W_rel onto the [U|W_rel] tail block
                        nc.tensor.matmul(
                            pg[:, DR : DR + NR],
                            _mm_dt(hrdT[:, 128 * lt : 128 * (lt + 1)], USE_F32R_G),
                            _mm_dt(w_rel, USE_F32R_G),
                            start=False,
                            stop=True,
                        )
                        pg_last = pg
                    # product with Hrd (broadcast over the r sub-blocks)
                    nseg = 5 if c < 9 else 1
                    hr = hrdn[:, lt, :]
                    hr_b = bass.AP(
                        tensor=hr.tensor,
                        offset=hr.offset,
                        ap=[hr.ap[0], [0, nseg], list(hr.ap[1])],
                    )
                    nc.vector.tensor_tensor(
                        out=prod[:, 5 * c : 5 * c + nseg, :],
                        in0=pg[:, 0 : nseg * DR].rearrange(
                            "p (s j) -> p s j", s=nseg
                        ),
                        in1=hr_b,
                        op=mybir.AluOpType.mult,
                    )
                red = workp1.tile([128, NR], F32, tag="red")
                nc.vector.tensor_reduce(
                    red, prod, axis=mybir.AxisListType.X, op=mybir.AluOpType.add
                )
                nc.vector.tensor_tensor(
                    out=srel_sb[:, lt, :],
                    in0=red,
                    in1=pg_last[:, DR : DR + NR],
                    op=mybir.AluOpType.add,
                )
                nc.sync.dma_start(
                    out=srel_o[b, 128 * lt : 128 * (lt + 1), :], in_=srel_sb[:, lt, :]
                )

    return nc


def prep_weights(W_ah, b_ah, W_ad, b_ad, W_rh, b_rh, W_rd, b_rd, W_arc, b_arc,
                 U_rel, W_rel, b_rel):
    f = np.float32
    wahT = np.concatenate([W_ah.T, b_ah[None, :]], axis=0).astype(f)  # [801, 500]
    wadT = np.concatenate([W_ad.T, b_ad[None, :]], axis=0).astype(f)
    wrhT = np.concatenate([W_rh.T, b_rh[None, :]], axis=0).astype(f)  # [801, 100]
    wrdT = np.concatenate([W_rd.T, b_rd[None, :]], axis=0).astype(f)
    warc = np.concatenate([W_arc, b_arc[None, :]], axis=0).astype(f)  # [501, 500]
    u_flat = U_rel.reshape(DR, NR * DR)  # [i, (r, j)]
    top = np.concatenate([u_flat, W_rel], axis=1)  # [100, 4646]
    bot = np.concatenate([np.zeros((1, NR * DR), f), b_rel[None, :].astype(f)], axis=1)
    uaug = np.concatenate([top, bot], axis=0).astype(f)  # [101, 4646]
    return dict(wahT=np.ascontiguousarray(wahT), wadT=np.ascontiguousarray(wadT),
                wrhT=np.ascontiguousarray(wrhT), wrdT=np.ascontiguousarray(wrdT),
                warc=np.ascontiguousarray(warc), uaug=np.ascontiguousarray(uaug),
                wrel=np.ascontiguousarray(W_rel.astype(f)))


_nc_cache = None


def get_nc():
    global _nc_cache
    if _nc_cache is None:
        _nc_cache = build_nc()
    return _nc_cache


def make_in_maps(H, weights):
    H = np.ascontiguousarray(np.asarray(H, dtype=np.float32))
    in_maps = []
    for c in range(NCORES):
        m = {"H": H[c * BSH : (c + 1) * BSH]}
        m.update(weights)
        in_maps.append(m)
    return in_maps


def run_spmd(H, weights, trace=False, **kw):
    nc = get_nc()
    res = run_bass_kernel_spmd(
        nc, make_in_maps(H, weights), list(range(NCORES)), trace=trace, **kw
    )
    s_arc = np.concatenate([r["S_arc"] for r in res.results], axis=0)
    s_rel = np.concatenate([r["S_rel"] for r in res.results], axis=0)
    preds = np.concatenate([r["preds"] for r in res.results], axis=0).astype(np.int32)
    return (s_arc, s_rel, preds), res


def kernel(H, sent_lens=None, **w):
    weights = prep_weights(
        w["W_ah"], w["b_ah"], w["W_ad"], w["b_ad"], w["W_rh"], w["b_rh"],
        w["W_rd"], w["b_rd"], w["W_arc"], w["b_arc"], w["U_rel"], w["W_rel"],
        w["b_rel"],
    )
    outs, _ = run_spmd(H, weights, trace=False)
    return outs


# revision 11
# speedup vs baseline: 20.0858x; 20.0858x over previous
"""BiAffineAttention Trainium2 kernel (8-core data-parallel over batch).

Math per batch element b (from the reference):
  Hah = relu(H @ W_ah^T + b_ah)        [L, DA]   (arc head)
  Had = relu(H @ W_ad^T + b_ad)        [L, DA]   (arc dep)
  Hrh = relu(H @ W_rh^T + b_rh)        [L, DR]   (rel head)
  Hrd = relu(H @ W_rd^T + b_rd)        [L, DR]   (rel dep)
  S_arc[l, m] = sum_e (Had @ W_arc + b_arc)[l, e] * Hah[m, e]
  preds = argmax_m S_arc
  Hg[l] = Hrh[preds[l]]
  S_rel[l, r] = sum_ij Hg[l,i] U[i,r,j] Hrd[l,j] + (Hg+Hrd)@W_rel + b_rel

On-chip layout strategy (everything chained through transposed activations):
  - H^T [HID2+1, L] built once per b via PE transposes (last row = ones for
    folding projection biases into the matmul via augmented weights).
  - Projections computed as P^T = W_aug^T.T @ H^T (transposed layout) except
    Hrh which is needed in natural layout (lhsT for the gather matmul).
  - A'^T = W_arc_aug.T @ Had^T_aug (b_arc folded via ones row).
  - S_arc tile [l, m] = A'^T.T @ Hah^T; argmax over free dim (m).
  - Gather Hg^T = Hrh_nat.T @ OneHotT where OneHotT[m, l] = (preds[l] == m),
    built from a broadcast matmul of the preds row + iota compare.
  - G = Hg_aug^T.T @ [U_flat | W_rel; 0 | b_rel]  (K = DR+1, ones row in Hg)
  - S_rel = segmented_reduce_j(G[:, (r, j)] * Hrd_nat[:, j]) + G_wrel_block
            + Hrd @ W_rel (accumulated into the same PSUM block).
"""

import os
import sys
from contextlib import ExitStack

import numpy as np

import concourse.bass as bass
import concourse.bacc as bacc
import concourse.tile as tile
from concourse import mybir
from concourse.bass_utils import run_bass_kernel_spmd
from concourse.masks import make_identity

F32 = mybir.dt.float32
F32R = mybir.dt.float32r
I32 = mybir.dt.int32
U32 = mybir.dt.uint32

B, L, HID2 = 64, 512, 800
DA, DR, NR = 500, 100, 46
NCORES = 8
BSH = B // NCORES  # batch shard per core

KH = HID2 + 1      # augmented contraction dim for projections (ones row)
KA = DA + 1        # augmented contraction dim for W_arc
KG = DR + 1        # augmented contraction dim for the G matmul
NG = NR * DR + NR  # 4646: [U_flat | W_rel(+b_rel)] columns

# dtype config: float32r streams fp32 bits through the PE at ~4x the rate of
# plain fp32 (which needs 2 half-speed passes). Numerics validated on HW.
USE_F32R_PROJ = False  # projections + A + S_arc (argmax-critical chain)
USE_F32R_G = False     # G matmul + gather matmul
DEBUG_DUMPS = False
REPEAT = 1  # bench: repeat the whole batch loop N times inside the kernel

_chunks_cache = {}


def _chunks(total, step=128):
    key = (total, step)
    if key not in _chunks_cache:
        out = []
        o = 0
        while o < total:
            out.append((o, min(step, total - o)))
            o += step
        _chunks_cache[key] = out
    return _chunks_cache[key]


def _mm_dt(ap, use_f32r):
    return ap.bitcast(F32R) if use_f32r else ap


def build_nc():
    nc = bacc.Bacc()

    H = nc.declare_dram_parameter("H", [BSH, L, HID2], F32, isOutput=False)
    wahT = nc.declare_dram_parameter("wahT", [KH, DA], F32, isOutput=False)
    wadT = nc.declare_dram_parameter("wadT", [KH, DA], F32, isOutput=False)
    wrhT = nc.declare_dram_parameter("wrhT", [KH, DR], F32, isOutput=False)
    wrdT = nc.declare_dram_parameter("wrdT", [KH, DR], F32, isOutput=False)
    warc = nc.declare_dram_parameter("warc", [DA, DA], F32, isOutput=False)
    barc = nc.declare_dram_parameter("barc", [1, DA], F32, isOutput=False)
    brel = nc.declare_dram_parameter("brel", [1, NR], F32, isOutput=False)
    uaug = nc.declare_dram_parameter("uaug", [DR, NG], F32, isOutput=False)
    wrel = nc.declare_dram_parameter("wrel", [DR, NR], F32, isOutput=False)

    sarc_o = nc.declare_dram_parameter("S_arc", [BSH, L, L], F32, isOutput=True)
    srel_o = nc.declare_dram_parameter("S_rel", [BSH, L, NR], F32, isOutput=True)
    preds_o = nc.declare_dram_parameter("preds", [BSH, L], I32, isOutput=True)
    if DEBUG_DUMPS:
        dbg_hgT = nc.declare_dram_parameter("dbg_hgT", [DR, L], F32, isOutput=True)
        dbg_oh = nc.declare_dram_parameter("dbg_oh", [L, L], F32, isOutput=True)
        dbg_prod = nc.declare_dram_parameter("dbg_prod", [4, 128, NR, DR], F32, isOutput=True)
        dbg_hrh = nc.declare_dram_parameter("dbg_hrh", [128, 4, DR], F32, isOutput=True)
        dbg_hrdn = nc.declare_dram_parameter("dbg_hrdn", [128, 4, DR], F32, isOutput=True)
        dbg_hrdT = nc.declare_dram_parameter("dbg_hrdT", [DR, L], F32, isOutput=True)

    with tile.TileContext(nc) as tc, ExitStack() as ctx:
        consts = ctx.enter_context(tc.tile_pool(name="consts", bufs=1))
        wpool = ctx.enter_context(tc.tile_pool(name="wpool", bufs=1))
        hstage = ctx.enter_context(tc.tile_pool(name="hstage", bufs=2))
        htp = ctx.enter_context(tc.tile_pool(name="htp", bufs=2))
        projp = ctx.enter_context(tc.tile_pool(name="projp", bufs=2))
        projp1 = ctx.enter_context(tc.tile_pool(name="projp1", bufs=1))
        workp = ctx.enter_context(tc.tile_pool(name="workp", bufs=2))
        workp1 = ctx.enter_context(tc.tile_pool(name="workp1", bufs=1))
        outp = ctx.enter_context(tc.tile_pool(name="outp", bufs=2))
        ppA = ctx.enter_context(tc.tile_pool(name="ppA", bufs=3, space="PSUM"))
        ppG = ctx.enter_context(tc.tile_pool(name="ppG", bufs=2, space="PSUM"))
        ppT = ctx.enter_context(tc.tile_pool(name="ppT", bufs=2, space="PSUM"))
        ppS = ctx.enter_context(tc.tile_pool(name="ppS", bufs=1, space="PSUM"))

        identity = consts.tile([128, 128], F32)
        make_identity(nc, identity)
        iota_i = consts.tile([128, 1], I32)
        nc.gpsimd.iota(iota_i, pattern=[[0, 1]], base=0, channel_multiplier=1)
        iota_f = consts.tile([128, 1], F32)
        nc.vector.tensor_copy(iota_f, iota_i)
        ones_col = consts.tile([1, 128], F32)
        nc.vector.memset(ones_col, 1.0)
        ones_row = consts.tile([1, L], F32)
        nc.vector.memset(ones_row, 1.0)

        # -------- weights (resident) --------
        w_ahT = wpool.tile([128, 7, DA], F32)
        w_adT = wpool.tile([128, 7, DA], F32)
        w_rhT = wpool.tile([128, 7, DR], F32)
        w_rdT = wpool.tile([128, 7, DR], F32)
        for k, (o, sz) in enumerate(_chunks(KH)):
            nc.sync.dma_start(out=w_ahT[0:sz, k, :], in_=wahT[o : o + sz, :])
            nc.sync.dma_start(out=w_adT[0:sz, k, :], in_=wadT[o : o + sz, :])
            nc.sync.dma_start(out=w_rhT[0:sz, k, :], in_=wrhT[o : o + sz, :])
            nc.sync.dma_start(out=w_rdT[0:sz, k, :], in_=wrdT[o : o + sz, :])
        w_arc = wpool.tile([128, 4, DA], F32)
        for k, (o, sz) in enumerate(_chunks(DA)):
            nc.sync.dma_start(out=w_arc[0:sz, k, :], in_=warc[o : o + sz, :])
        b_arc_sb = wpool.tile([1, DA], F32)
        nc.sync.dma_start(out=b_arc_sb, in_=barc[:, :])
        b_rel_sb = wpool.tile([1, NR], F32)
        nc.sync.dma_start(out=b_rel_sb, in_=brel[:, :])
        u_aug = wpool.tile([DR, NG], F32)
        nc.sync.dma_start(out=u_aug, in_=uaug[:, :])
        w_rel = wpool.tile([DR, NR], F32)
        nc.sync.dma_start(out=w_rel, in_=wrel[:, :])

        relu = mybir.ActivationFunctionType.Relu

        for b in [bb % BSH for bb in range(REPEAT * BSH)]:
            # -------- stage 1: load H[b], build H^T (+ones row) --------
            ht = htp.tile([128, 7, L], F32, tag="ht")
            for lt in range(4):
                hn = hstage.tile([128, HID2], F32, tag="hn")
                nc.sync.dma_start(out=hn, in_=H[b, 128 * lt : 128 * (lt + 1), :])
                for k, (o, sz) in enumerate(_chunks(HID2)):
                    ptr = ppT.tile([128, 128], F32, tag="ptr")
                    nc.tensor.transpose(ptr[0:sz, :], hn[:, o : o + sz], identity)
                    nc.scalar.copy(
                        out=ht[0:sz, k, 128 * lt : 128 * (lt + 1)], in_=ptr[0:sz, :]
                    )
            nc.vector.memset(ht[32:33, 6, :], 1.0)  # ones row (d = 800)

            # -------- stage 2: projections --------
            hahT = projp.tile([128, 4, L], F32, tag="hahT")
            hadT = projp.tile([128, 4, L], F32, tag="hadT")
            for w_sb, dst in ((w_ahT, hahT), (w_adT, hadT)):
                for m, (mo, msz) in enumerate(_chunks(DA)):
                    pr = ppA.tile([128, L], F32, tag="pA")
                    for k, (o, sz) in enumerate(_chunks(KH)):
                        nc.tensor.matmul(
                            pr[0:msz, :],
                            _mm_dt(w_sb[0:sz, k, mo : mo + msz], USE_F32R_PROJ),
                            _mm_dt(ht[0:sz, k, :], USE_F32R_PROJ),
                            start=(k == 0),
                            stop=(k == 6),
                        )
                    nc.scalar.activation(dst[0:msz, m, :], pr[0:msz, :], relu)

            # Hrd^T [DR, L]
            hrdT = projp.tile([DR, L], F32, tag="hrdT")
            prd = ppA.tile([128, L], F32, tag="pA")
            for k, (o, sz) in enumerate(_chunks(KH)):
                nc.tensor.matmul(
                    prd[0:DR, :],
                    _mm_dt(w_rdT[0:sz, k, :], USE_F32R_PROJ),
                    _mm_dt(ht[0:sz, k, :], USE_F32R_PROJ),
                    start=(k == 0),
                    stop=(k == 6),
                )
            nc.scalar.activation(hrdT, prd[0:DR, :], relu)

            # Hrh natural [l, i] per l-tile
            hrh = projp.tile([128, 4, DR], F32, tag="hrh")
            for lt in range(4):
                prh = ppS.tile([128, DR], F32, tag="pS")
                for k, (o, sz) in enumerate(_chunks(KH)):
                    nc.tensor.matmul(
                        prh,
                        _mm_dt(ht[0:sz, k, 128 * lt : 128 * (lt + 1)], USE_F32R_PROJ),
                        _mm_dt(w_rhT[0:sz, k, :], USE_F32R_PROJ),
                        start=(k == 0),
                        stop=(k == 6),
                    )
                nc.scalar.activation(hrh[:, lt, :], prh, relu)

            # Hrd natural via PE transpose of Hrd^T
            hrdn = projp.tile([128, 4, DR], F32, tag="hrdn")
            for lt in range(4):
                ptr2 = ppT.tile([128, 128], F32, tag="ptr")
                nc.tensor.transpose(
                    ptr2[:, 0:DR], hrdT[:, 128 * lt : 128 * (lt + 1)], identity[0:DR, 0:DR]
                )
                nc.scalar.copy(out=hrdn[:, lt, :], in_=ptr2[:, 0:DR])

            if DEBUG_DUMPS and b == 0:
                nc.sync.dma_start(out=dbg_hrh[:, :, :], in_=hrh)
                nc.sync.dma_start(out=dbg_hrdn[:, :, :], in_=hrdn)
                nc.sync.dma_start(out=dbg_hrdT[:, :], in_=hrdT)
            # -------- stage 3: A'^T = W_arc_aug.T @ Had^T_aug --------
            aT = projp1.tile([128, 4, L], F32, tag="aT")
            for m, (mo, msz) in enumerate(_chunks(DA)):
                pa = ppA.tile([128, L], F32, tag="pA")
                for k, (o, sz) in enumerate(_chunks(DA)):
                    nc.tensor.matmul(
                        pa[0:msz, :],
                        _mm_dt(w_arc[0:sz, k, mo : mo + msz], USE_F32R_PROJ),
                        _mm_dt(hadT[0:sz, k, :], USE_F32R_PROJ),
                        start=(k == 0),
                        stop=False,
                    )
                nc.tensor.matmul(
                    pa[0:msz, :],
                    _mm_dt(b_arc_sb[0:1, mo : mo + msz], USE_F32R_PROJ),
                    _mm_dt(ones_row, USE_F32R_PROJ),
                    start=False,
                    stop=True,
                )
                nc.scalar.copy(out=aT[0:msz, m, :], in_=pa[0:msz, :])

            # -------- stage 4: S_arc + argmax --------
            sarc_sb = outp.tile([128, 4, L], F32, tag="sarc")
            mx = workp1.tile([128, 4, 8], F32, tag="mx")
            mi = workp1.tile([128, 4, 8], U32, tag="mi")
            pred_f = workp1.tile([128, 4], F32, tag="predf")
            pred_i = workp1.tile([128, 4], I32, tag="predi")
            for lt in range(4):
                ps = ppA.tile([128, L], F32, tag="pA")
                for k, (o, sz) in enumerate(_chunks(DA)):
                    nc.tensor.matmul(
                        ps,
                        _mm_dt(aT[0:sz, k, 128 * lt : 128 * (lt + 1)], USE_F32R_PROJ),
                        _mm_dt(hahT[0:sz, k, :], USE_F32R_PROJ),
                        start=(k == 0),
                        stop=(k == 3),
                    )
                nc.scalar.copy(out=sarc_sb[:, lt, :], in_=ps)
                nc.sync.dma_start(
                    out=sarc_o[b, 128 * lt : 128 * (lt + 1), :], in_=sarc_sb[:, lt, :]
                )
                nc.vector.max(out=mx[:, lt, :], in_=sarc_sb[:, lt, :])
                nc.vector.max_index(mi[:, lt, :], mx[:, lt, :], sarc_sb[:, lt, :])
                nc.vector.tensor_copy(pred_i[:, lt : lt + 1], mi[:, lt, 0:1])
                nc.vector.tensor_copy(pred_f[:, lt : lt + 1], mi[:, lt, 0:1])
                nc.sync.dma_start(
                    out=preds_o[b, 128 * lt : 128 * (lt + 1)],
                    in_=pred_i[:, lt : lt + 1],
                )

            # -------- stage 5: gather Hg^T via one-hot matmul --------
            head_row = workp1.tile([1, L], F32, tag="hrow")
            for lt in range(4):
                prow = ppS.tile([1, 128], F32, tag="pS")
                nc.tensor.transpose(prow, pred_f[:, lt : lt + 1], identity)
                nc.scalar.copy(
                    out=head_row[0:1, 128 * lt : 128 * (lt + 1)], in_=prow
                )
            pbc = ppA.tile([128, L], F32, tag="pA")
            nc.tensor.matmul(pbc, ones_col, head_row, start=True, stop=True)
            oh = workp1.tile([128, 4, L], F32, tag="oh")
            for mt in range(4):
                nc.vector.tensor_scalar(
                    out=oh[:, mt, :],
                    in0=pbc,
                    scalar1=iota_f,
                    scalar2=float(128 * mt),
                    op0=mybir.AluOpType.subtract,
                    op1=mybir.AluOpType.is_equal,
                )
            phg = ppS.tile([DR, L], F32, tag="pS")
            for mt in range(4):
                nc.tensor.matmul(
                    phg,
                    _mm_dt(hrh[:, mt, :], USE_F32R_G),
                    _mm_dt(oh[:, mt, :], USE_F32R_G),
                    start=(mt == 0),
                    stop=(mt == 3),
                )
            hgT = workp1.tile([DR, L], F32, tag="hgT")
            nc.scalar.copy(out=hgT, in_=phg)
            if DEBUG_DUMPS and b == 0:
                nc.sync.dma_start(out=dbg_hgT[:, :], in_=hgT)
                for mt in range(4):
                    nc.sync.dma_start(
                        out=dbg_oh[128 * mt : 128 * (mt + 1), :], in_=oh[:, mt, :]
                    )

            # -------- stage 6: G matmul + bilinear reduce + S_rel --------
            srel_sb = outp.tile([128, 4, NR], F32, tag="srel")
            for lt in range(4):
                prod = workp1.tile([128, NR, DR], F32, tag="prod")
                pg_last = None
                for c in range(10):
                    n0 = 500 * c
                    nw = 500 if c < 9 else NG - 4500  # 146 for the last chunk
                    pg = ppG.tile([128, 500], F32, tag="pg")
                    nc.tensor.matmul(
                        pg[:, 0:nw],
                        _mm_dt(hgT[:, 128 * lt : 128 * (lt + 1)], USE_F32R_G),
                        _mm_dt(u_aug[:, n0 : n0 + nw], USE_F32R_G),
                        start=True,
                        stop=(c < 9),
                    )
                    if c == 9:
                        # accumulate Hrd @ W_rel + b_rel onto the [U|W_rel] tail
                        nc.tensor.matmul(
                            pg[:, DR : DR + NR],
                            _mm_dt(hrdT[:, 128 * lt : 128 * (lt + 1)], USE_F32R_G),
                            _mm_dt(w_rel, USE_F32R_G),
                            start=False,
                            stop=False,
                        )
                        nc.tensor.matmul(
                            pg[:, DR : DR + NR],
                            _mm_dt(ones_row[0:1, 0:128], USE_F32R_G),
                            _mm_dt(b_rel_sb, USE_F32R_G),
                            start=False,
                            stop=True,
                        )
                        pg_last = pg
                    # product with Hrd (broadcast over the r sub-blocks)
                    nseg = 5 if c < 9 else 1
                    hr = hrdn[:, lt, :]
                    hr_b = bass.AP(
                        tensor=hr.tensor,
                        offset=hr.offset,
                        ap=[hr.ap[0], [0, nseg], list(hr.ap[1])],
                    )
                    nc.vector.tensor_tensor(
                        out=prod[:, 5 * c : 5 * c + nseg, :],
                        in0=pg[:, 0 : nseg * DR].rearrange(
                            "p (s j) -> p s j", s=nseg
                        ),
                        in1=hr_b,
                        op=mybir.AluOpType.mult,
                    )
                if DEBUG_DUMPS and b == 0:
                    nc.sync.dma_start(out=dbg_prod[lt], in_=prod)
                red = workp1.tile([128, NR], F32, tag="red")
                nc.vector.tensor_reduce(
                    red, prod, axis=mybir.AxisListType.X, op=mybir.AluOpType.add
                )
                nc.vector.tensor_tensor(
                    out=srel_sb[:, lt, :],
                    in0=red,
                    in1=pg_last[:, DR : DR + NR],
                    op=mybir.AluOpType.add,
                )
                nc.sync.dma_start(
                    out=srel_o[b, 128 * lt : 128 * (lt + 1), :], in_=srel_sb[:, lt, :]
                )

    nc.compile()
    return nc


def prep_weights(W_ah, b_ah, W_ad, b_ad, W_rh, b_rh, W_rd, b_rd, W_arc, b_arc,
                 U_rel, W_rel, b_rel):
    f = np.float32
    wahT = np.concatenate([W_ah.T, b_ah[None, :]], axis=0).astype(f)  # [801, 500]
    wadT = np.concatenate([W_ad.T, b_ad[None, :]], axis=0).astype(f)
    wrhT = np.concatenate([W_rh.T, b_rh[None, :]], axis=0).astype(f)  # [801, 100]
    wrdT = np.concatenate([W_rd.T, b_rd[None, :]], axis=0).astype(f)
    warc = np.asarray(W_arc, f)  # [500, 500]
    u_flat = U_rel.reshape(DR, NR * DR)  # [i, (r, j)]
    uaug = np.concatenate([u_flat, W_rel], axis=1).astype(f)  # [100, 4646]
    return dict(wahT=np.ascontiguousarray(wahT), wadT=np.ascontiguousarray(wadT),
                wrhT=np.ascontiguousarray(wrhT), wrdT=np.ascontiguousarray(wrdT),
                warc=np.ascontiguousarray(warc), uaug=np.ascontiguousarray(uaug),
                wrel=np.ascontiguousarray(W_rel.astype(f)),
                barc=np.ascontiguousarray(b_arc.reshape(1, DA).astype(f)),
                brel=np.ascontiguousarray(b_rel.reshape(1, NR).astype(f)))


_nc_cache = None


def get_nc():
    global _nc_cache
    if _nc_cache is None:
        _nc_cache = build_nc()
    return _nc_cache


def make_in_maps(H, weights):
    H = np.ascontiguousarray(np.asarray(H, dtype=np.float32))
    in_maps = []
    for c in range(NCORES):
        m = {"H": H[c * BSH : (c + 1) * BSH]}
        m.update(weights)
        in_maps.append(m)
    return in_maps


def run_spmd(H, weights, trace=False, **kw):
    nc = get_nc()
    res = run_bass_kernel_spmd(
        nc, make_in_maps(H, weights), list(range(NCORES)), trace=trace, **kw
    )
    s_arc = np.concatenate([r["S_arc"] for r in res.results], axis=0)
    s_rel = np.concatenate([r["S_rel"] for r in res.results], axis=0)
    preds = np.concatenate([r["preds"] for r in res.results], axis=0).astype(np.int32)
    return (s_arc, s_rel, preds), res


def kernel(H, sent_lens=None, **w):
    weights = prep_weights(
        w["W_ah"], w["b_ah"], w["W_ad"], w["b_ad"], w["W_rh"], w["b_rh"],
        w["W_rd"], w["b_rd"], w["W_arc"], w["b_arc"], w["U_rel"], w["W_rel"],
        w["b_rel"],
    )
    outs, _ = run_spmd(H, weights, trace=False)
    return outs
